# revision 1
# baseline (speedup 1.0000x reference)
"""AWQ linear kernel for Trainium2, 8-core tensor/data-parallel SPMD.

Computes out = x @ (weight * weight_scale).T + bias with
  x:[4,2048,4096] f32, weight:[4096,4096] int32 (int8-valued),
  weight_scale:[4096,1] f32, bias:[4096] f32.

Sharding: tokens (B*S=8192) split 2-way, out-features split 4-way
-> 8 cores, each computing a [4096, 1024] f32 output shard. No
cross-core communication.

Per-core plan (all engines overlapped by the Tile scheduler):
 - W: SWDGE cast-DMA int32 -> bf16 (int8-valued weights are bf16-exact),
   ACT dequant by per-row scale, PE transpose via identity, DVE copy ->
   resident W^T [in,out] (8.4MB SBUF). All during the pipeline head.
 - x: SWDGE cast-DMA f32 -> bf16 into a row-major DRAM scratch (512-token
   chunks, pool-gated 4 ahead), then DMA-xbar-transpose loads on the sync
   HWDGE ring produce x^T [in,tok] tiles. Transposes stay on ONE ring:
   concurrent xbar transposes from both HWDGE rings corrupt data (HW bug).
 - TensorE: 2048 bf16 matmuls [K=128,M=128]x[K=128,N=512] accumulating
   fp32 in PSUM over K=4096.
 - VectorE: psum + bias (pre-broadcast across partitions via a K=1
   fp32 matmul against ones) -> SBUF f32 -> out-stores on the SWDGE ring.
"""

import contextlib

import numpy as np

import concourse.bass as bass
import concourse.tile as tile
import concourse.mybir as mybir
from concourse import bacc
from concourse.bass_utils import run_bass_kernel_spmd

P = 128

# full problem
B, S = 4, 2048
IN_F = 4096
OUT_F = 4096
TOK_SHARDS = 2   # token halves
OUT_SHARDS = 4   # out-feature quarters
N_CORES = TOK_SHARDS * OUT_SHARDS

# per-core shard
TOK = (B * S) // TOK_SHARDS     # 4096
OUTF = OUT_F // OUT_SHARDS      # 1024
CHUNK = 512                     # tokens per x pipeline chunk
XT_BUFS = 2


def build_nc(tok=TOK, in_f=IN_F, outf=OUTF, chunk=CHUNK, x_mode="bitcast"):
    kc_n = in_f // P            # k chunks of 128
    # token chunk sizes: small leading chunks so the x pipeline bootstraps
    # while the W path (load->dequant->PE-transpose) is still in flight
    csizes = []
    rem = tok
    for s in (P, P, 2 * P):
        if rem - s >= chunk:
            csizes.append(s)
            rem -= s
    csizes += [chunk] * (rem // chunk)
    assert sum(csizes) == tok
    nch = len(csizes)
    coffs = [sum(csizes[:i]) for i in range(nch)]
    nhw = min(512, outf)        # matmul free dim
    nnh = outf // nhw           # n tiles per output row block
    wrow_n = outf // P          # weight row chunks of 128

    nc = bacc.Bacc("TRN2", target_bir_lowering=False, debug=False,
                   num_devices=N_CORES)
    x_h = nc.dram_tensor("x", [tok, in_f], mybir.dt.float32,
                         kind="ExternalInput").ap()
    w_h = nc.dram_tensor("weight", [outf, in_f], mybir.dt.int32,
                         kind="ExternalInput").ap()
    ws_h = nc.dram_tensor("weight_scale", [outf, 1], mybir.dt.float32,
                          kind="ExternalInput").ap()
    b_h = nc.dram_tensor("bias", [1, outf], mybir.dt.float32,
                         kind="ExternalInput").ap()
    out_h = nc.dram_tensor("out", [tok, outf], mybir.dt.float32,
                           kind="ExternalOutput").ap()

    with tile.TileContext(nc) as tc, contextlib.ExitStack() as ctx:
        dram_pool = ctx.enter_context(tc.tile_pool(name="dram", bufs=1, space="DRAM"))
        wt_pool = ctx.enter_context(tc.tile_pool(name="wt", bufs=1))
        const_pool = ctx.enter_context(tc.tile_pool(name="const", bufs=1))
        wprep_pool = ctx.enter_context(tc.tile_pool(name="wprep", bufs=2))
        xstage_pool = ctx.enter_context(tc.tile_pool(name="xstage", bufs=2))
        xt_pool = ctx.enter_context(tc.tile_pool(name="xt", bufs=2))
        out_pool = ctx.enter_context(tc.tile_pool(name="outp", bufs=4))
        psum_pool = ctx.enter_context(tc.tile_pool(name="psum", bufs=8, space="PSUM"))

        # bias broadcast across partitions: psum[p, n] = ones[1, p].T @ bias[1, n]
        bias_sb = const_pool.tile([1, outf], mybir.dt.float32)
        nc.scalar.dma_start(bias_sb, b_h)
        ones = const_pool.tile([1, P], mybir.dt.float32)
        nc.vector.memset(ones, 1.0)
        bias_rep = const_pool.tile([P, outf], mybir.dt.float32)
        for nh in range(nnh):
            pb = psum_pool.tile([P, nhw], mybir.dt.float32, tag="ps")
            nc.tensor.matmul(pb, ones, bias_sb[:, nh * nhw:(nh + 1) * nhw],
                             start=True, stop=True)
            nc.vector.tensor_copy(out=bias_rep[:, nh * nhw:(nh + 1) * nhw], in_=pb)

        ident = const_pool.tile([P, P], mybir.dt.bfloat16)
        from concourse.masks import make_identity
        make_identity(nc, ident)

        # ---- W path, all on-chip ----
        # SWDGE cast-load int32 -> bf16 (int8-valued weights are bf16-exact),
        # ACT dequant by per-row scale, PE transpose via identity, DVE copy
        # into the resident W^T.
        wt = wt_pool.tile([P, kc_n, outf], mybir.dt.bfloat16)
        scs = []
        for wc in range(wrow_n):
            sc = wprep_pool.tile([P, 1], mybir.dt.float32, tag="sc", bufs=wrow_n)
            nc.gpsimd.dma_start(sc, ws_h[wc * P:(wc + 1) * P, :])
            scs.append(sc)
        wraws = []
        for wc in range(wrow_n):
            w_raw = wprep_pool.tile([P, in_f], mybir.dt.bfloat16, tag="wraw",
                                    bufs=3)
            nc.gpsimd.dma_start(w_raw, w_h[wc * P:(wc + 1) * P, :])
            wraws.append(w_raw)
        TGRP = min(8, kc_n)  # transposes batched per psum tile
        for wc in range(wrow_n):
            w_bf = wprep_pool.tile([P, in_f], mybir.dt.bfloat16, tag="wbf")
            nc.scalar.mul(w_bf, wraws[wc], scs[wc])
            for g in range(kc_n // TGRP):
                ptr = psum_pool.tile([P, TGRP * P], mybir.dt.bfloat16,
                                     tag="ps")
                for j in range(TGRP):
                    nc.tensor.transpose(
                        ptr[:, j * P:(j + 1) * P],
                        w_bf[:, (g * TGRP + j) * P:(g * TGRP + j + 1) * P],
                        ident)
                kta = g * TGRP
                nc.vector.tensor_copy(
                    out=wt[:, kta:kta + TGRP, wc * P:(wc + 1) * P],
                    in_=ptr.rearrange("p (t q) -> p t q", t=TGRP))

        # ---- main pipeline over token chunks ----
        # x path per m-block: SWDGE cast-load f32->bf16 row-major into SBUF
        # (cheap 16KB descriptors), HWDGE store into a kc-BLOCKED DRAM
        # scratch ([kc][t, 128] contiguous), then each xbar transpose reads
        # one fully contiguous region (4KB-concat M2S descriptors).
        for c in range(nch):
            csz = csizes[c]
            xbf = dram_pool.tile([csz, in_f], mybir.dt.bfloat16, tag="xbf",
                                 bufs=nch, padded_shape=[chunk, in_f])
            nc.gpsimd.dma_start(xbf, x_h[coffs[c]:coffs[c] + csz, :])
            xt = xt_pool.tile([P, kc_n, csz], mybir.dt.bfloat16, tag="xt",
                              bufs=XT_BUFS, padded_shape=[P, kc_n, chunk])
            for kc in range(kc_n):
                nc.sync.dma_start(xt[:, kc, :], xbf[:, kc * P:(kc + 1) * P],
                                  transpose=True)
            for m in range(csz // P):
                row0 = coffs[c] + m * P
                out_sb = out_pool.tile([P, outf], mybir.dt.float32, tag="osb",
                                       bufs=3)
                for nh in range(nnh):
                    ps = psum_pool.tile([P, nhw], mybir.dt.float32, tag="ps")
                    for kc in range(kc_n):
                        nc.tensor.matmul(
                            ps,
                            xt[:, kc, m * P:(m + 1) * P],
                            wt[:, kc, nh * nhw:(nh + 1) * nhw],
                            start=(kc == 0), stop=(kc == kc_n - 1))
                    nc.vector.tensor_add(out=out_sb[:, nh * nhw:(nh + 1) * nhw],
                                         in0=ps,
                                         in1=bias_rep[:, nh * nhw:(nh + 1) * nhw])
                # scalar ring: idle after the head, and crucially NOT the
                # gpsimd ring, where stores would queue behind the slot-gated
                # casts and stall MM drains via osb/psum back-pressure.
                nc.scalar.dma_start(out_h[row0:row0 + P, :], out_sb)
    nc.compile()
    return nc


def shard_inputs(x, weight, weight_scale, bias):
    xf = np.ascontiguousarray(x.reshape(B * S, IN_F))
    in_maps = []
    for core in range(N_CORES):
        r, q = divmod(core, OUT_SHARDS)
        in_maps.append({
            "x": np.ascontiguousarray(xf[r * TOK:(r + 1) * TOK]),
            "weight": np.ascontiguousarray(weight[q * OUTF:(q + 1) * OUTF]),
            "weight_scale": np.ascontiguousarray(weight_scale[q * OUTF:(q + 1) * OUTF]),
            "bias": np.ascontiguousarray(bias[q * OUTF:(q + 1) * OUTF]).reshape(1, OUTF),
        })
    return in_maps


def gather_outputs(results):
    halves = []
    for r in range(TOK_SHARDS):
        quarters = [results[r * OUT_SHARDS + q]["out"] for q in range(OUT_SHARDS)]
        halves.append(np.concatenate(quarters, axis=1))
    full = np.concatenate(halves, axis=0)
    return np.ascontiguousarray(full.reshape(B, S, OUT_F).astype(np.float32))


_NC_CACHE = {}

X_MODE = "scratch"


def _get_nc(x_mode=None):
    x_mode = x_mode or X_MODE
    if x_mode not in _NC_CACHE:
        _NC_CACHE[x_mode] = build_nc(x_mode=x_mode)
    return _NC_CACHE[x_mode]


def kernel(x, weight, weight_scale, bias, _trace=False, _x_mode=None):
    nc = _get_nc(_x_mode)
    in_maps = shard_inputs(np.asarray(x), np.asarray(weight),
                           np.asarray(weight_scale), np.asarray(bias))
    res = run_bass_kernel_spmd(nc, in_maps, core_ids=list(range(N_CORES)),
                               trace=_trace)
    out = gather_outputs(res.results)
    if _trace:
        return out, res
    return out

